# revision 3
# baseline (speedup 1.0000x reference)
"""ChildSum TreeLSTM on 8 trn2 NeuronCores — subtree-forest partition.

Strategy: carve the tree into ~73 complete subtrees (size <= CAP) that
bin-pack onto 8 cores (~510 nodes each, full hidden dim per node, no
feature split). Each core runs its forest level-by-level fully locally
(gather children rows from per-core DRAM stores + one-hot S matmuls for
segment sums; U matmuls in bf16). The ~14 "residual" top nodes are
replicated on all cores: their subtree-root inputs cross cores via ONE
AllGather of [h|c|Uf.h] rows. Everything else is collective-free.

Precision: x/W/Wx/h/c/t stores fp32; U, the U-matmul rhs (hsum, h) and
the AllGather payload are bf16. End-to-end relerr ~1e-2 (tol 2e-2).
"""
import numpy as np

N = 4096
H = 1024
NCORES = 8
CAP = 384
NRESP = 16          # padded residual count
KCH = 8             # U contraction chunks
KCHX = 9            # W contraction chunks (incl bias row)


def _wrap_idx(a):
    """dma_gather index layout: idx[i] at [i%16, i//16], tiled to 128 parts."""
    a = np.asarray(a, np.int64)
    n = len(a)
    c = (n + 15) // 16
    w = np.zeros((16, c), np.int16)
    w[np.arange(n) % 16, np.arange(n) // 16] = a.astype(np.int16)
    return np.tile(w, (8, 1))


def _schedule(head):
    head = np.asarray(head).astype(np.int64)
    n = head.shape[0]
    size = np.ones(n + 1, np.int64)
    for j in range(n):
        size[head[j]] += size[j]

    resid = np.zeros(n, bool)
    unit_root = np.zeros(n, bool)
    for v in range(n):
        if size[v] <= CAP:
            p = head[v]
            if p == n or size[p] > CAP:
                unit_root[v] = True
        else:
            resid[v] = True
    units = np.where(unit_root)[0]
    usizes = size[units]
    bins = [0] * NCORES
    binunits = [[] for _ in range(NCORES)]
    for ui in np.argsort(-usizes):
        b = int(np.argmin(bins))
        bins[b] += usizes[ui]
        binunits[b].append(int(units[ui]))
    MAXROOTS = max(len(bu) for bu in binunits) + 1  # +1 spare zero slot
    assert MAXROOTS <= 16

    lev = np.zeros(n + 1, np.int64)
    for k in range(n):
        p = head[k]
        if p < n and not resid[k] and not resid[p]:
            if lev[p] < lev[k] + 1:
                lev[p] = lev[k] + 1
    uroot = np.full(n, -1, np.int64)
    for v in range(n - 1, -1, -1):
        if resid[v]:
            continue
        uroot[v] = v if unit_root[v] else uroot[head[v]]
    bin_of_unit = {}
    for b in range(NCORES):
        for u in binunits[b]:
            bin_of_unit[u] = b
    node_bin = np.array([bin_of_unit[uroot[v]] if not resid[v] else -1
                         for v in range(n)])

    nlev = int(lev[:n][~resid].max()) + 1
    percore_lists = []
    for b in range(NCORES):
        percore_lists.append([
            list(np.where((node_bin == b) & (lev[:n] == L) & ~resid)[0])
            for L in range(nlev)])
    widths = [max(len(percore_lists[b][L]) for b in range(NCORES))
              for L in range(nlev)]
    starts = np.concatenate([[0], np.cumsum(widths)]).astype(np.int64)
    NSLOT = int(starts[-1])

    # residual order: topological by residual sublevel
    rl = np.zeros(n + 1, np.int64)
    rr = sorted(np.where(resid)[0])
    for k in rr:
        p = head[k]
        if p < n and resid[p]:
            if rl[p] < rl[k] + 1:
                rl[p] = rl[k] + 1
    nsub = int(rl[:n][resid].max()) + 1 if len(rr) else 0
    sub_lists = [sorted(v for v in rr if rl[v] == s) for s in range(nsub)]
    resid_order = [v for s in range(nsub) for v in sub_lists[s]]
    NRES = len(resid_order)
    assert NRES <= NRESP
    ridx = {v: i for i, v in enumerate(resid_order)}
    resid_slot = {v: NSLOT + ridx[v] for v in resid_order}

    NCOLS = NSLOT + NRES
    PAD = NCOLS
    NROWS = NCOLS + 1
    W0 = widths[0]
    assert 256 <= W0 <= 512, W0
    assert 256 <= NCOLS - W0 <= 512, NCOLS - W0

    slot_of = []
    core_nodes = []
    for b in range(NCORES):
        m = {}
        arr = np.full(NSLOT, -1, np.int64)
        for L in range(nlev):
            for i, v in enumerate(percore_lists[b][L]):
                s = int(starts[L]) + i
                m[v] = s
                arr[s] = v
        slot_of.append(m)
        core_nodes.append(arr)

    kids = [[] for _ in range(n)]
    for k in range(n):
        p = head[k]
        if p < n:
            kids[p].append(k)

    # per-level gather plans (idx per core; S per core)
    idx_blocks = [[] for _ in range(NCORES)]   # concat axis=1
    ss_blocks = [[] for _ in range(NCORES)]
    icol = 0
    scol = 0
    levels = []
    for L in range(1, nlev):
        W = widths[L]
        maxch = max(sum(len(kids[p]) for p in percore_lists[b][L])
                    for b in range(NCORES))
        nch = max(1, -(-maxch // 128))
        for b in range(NCORES):
            ia = np.full(nch * 128, PAD, np.int64)
            ib = np.full(nch * 128, PAD, np.int64)
            S = np.zeros((128, nch * W), np.float32)
            pos = 0
            for i, p in enumerate(percore_lists[b][L]):
                for k in kids[p]:
                    ia[pos] = slot_of[b][k]
                    ib[pos] = slot_of[b][p]
                    S[pos % 128, (pos // 128) * W + i] = 1.0
                    pos += 1
            idx_blocks[b].append(_wrap_idx(ia))
            idx_blocks[b].append(_wrap_idx(ib))
            ss_blocks[b].append(S)
        levels.append(dict(L=L, W=W, s0=int(starts[L]), nch=nch,
                           iA=icol, iB=icol + nch * 8, sc=scol))
        icol += 2 * nch * 8
        scol += nch * W

    # roots idx (per core)
    iroots = icol
    for b in range(NCORES):
        ra = np.full(128, PAD, np.int64)
        for i, u in enumerate(binunits[b]):
            ra[i] = slot_of[b][u]
        idx_blocks[b].append(_wrap_idx(ra))
    icol += 8

    # AG slot map + a guaranteed-zero AG slot for padding
    agslot = {}
    for b in range(NCORES):
        for i, u in enumerate(binunits[b]):
            agslot[u] = b * MAXROOTS + i
    zslot = MAXROOTS - 1  # spare slot on core 0 (never a real root)

    # boundary pairs (resid parent, unit-root child)
    bpairs = [(int(head[v]), v) for v in range(n)
              if not resid[v] and head[v] < n and resid[head[v]]]
    assert len(bpairs) <= 128
    ipair = icol
    pa = np.full(128, zslot, np.int64)
    pb = np.full(128, PAD, np.int64)
    SP = np.zeros((128, NRESP), np.float32)
    for i, (p, k) in enumerate(bpairs):
        pa[i] = agslot[k]
        pb[i] = resid_slot[p]
        SP[i, ridx[p]] = 1.0
    for b in range(NCORES):
        idx_blocks[b].append(_wrap_idx(pa))
        idx_blocks[b].append(_wrap_idx(pb))
        ss_blocks[b].append(SP)
    spcol = scol
    scol += NRESP
    icol += 16

    # resid slots idx (for wxf transpose build)
    irs = icol
    rs = np.full(128, PAD, np.int64)
    for i, v in enumerate(resid_order):
        rs[i] = resid_slot[v]
    for b in range(NCORES):
        idx_blocks[b].append(_wrap_idx(rs))
    icol += 8

    # sublevel metadata
    subs = []
    r0 = 0
    for s in range(nsub):
        w = len(sub_lists[s])
        edges = []
        for v in sub_lists[s]:
            p = head[v]
            if p < n and resid[p]:
                edges.append((ridx[int(p)], ridx[v]))
        subs.append(dict(r0=r0, w=w, edges=edges))
        r0 += w

    idxt = [np.concatenate(idx_blocks[b], axis=1) for b in range(NCORES)]
    sst = [np.concatenate(ss_blocks[b], axis=1) for b in range(NCORES)]

    return dict(nlev=nlev, widths=widths, starts=starts, NSLOT=NSLOT,
                NRES=NRES, NCOLS=NCOLS, PAD=PAD, NROWS=NROWS, W0=W0,
                MAXROOTS=MAXROOTS,
                levels=levels, iroots=iroots, ipair=ipair, irs=irs,
                spcol=spcol, icols=icol, scols=scol, subs=subs, nsub=nsub,
                core_nodes=core_nodes, resid_order=resid_order,
                idxt=idxt, sst=sst)


def _build_nc(s):
    import concourse.mybir as mybir
    import concourse.tile as tile
    from concourse import bacc
    from concourse.masks import make_identity

    F32 = mybir.dt.float32
    F32R = mybir.dt.float32r
    BF16 = mybir.dt.bfloat16
    I16 = mybir.dt.int16
    SIG = mybir.ActivationFunctionType.Sigmoid
    TANH = mybir.ActivationFunctionType.Tanh

    nlev = s["nlev"]
    MAXROOTS = s["MAXROOTS"]
    W0 = s["W0"]
    NCOLS = s["NCOLS"]
    NROWS = s["NROWS"]
    NRES = s["NRES"]
    BCOLS = NCOLS - W0
    ROFF = s["NSLOT"] - W0   # resid col offset within sweep B

    nc = bacc.Bacc("TRN2", target_bir_lowering=False, debug=False,
                   num_devices=NCORES)
    xT = nc.declare_dram_parameter("xT", [KCHX * 128, NCOLS], F32R,
                                   isOutput=False)
    # WT columns are of-major: col block = of*512 + g*128 (g in iouf)
    WT = nc.declare_dram_parameter("WT", [KCHX * 128, 4096], F32R,
                                   isOutput=False)
    UT = nc.declare_dram_parameter("UT", [KCH * 128, 4096], BF16,
                                   isOutput=False)
    IDX = nc.declare_dram_parameter("IDX", [128, s["icols"]], I16,
                                    isOutput=False)
    SS = nc.declare_dram_parameter("SS", [128, s["scols"]], F32,
                                   isOutput=False)
    S16 = nc.declare_dram_parameter("S16", [128, s["scols"]], BF16,
                                    isOutput=False)
    h_out = nc.declare_dram_parameter("h_out", [H, NCOLS], F32, isOutput=True)
    c_out = nc.declare_dram_parameter("c_out", [H, NCOLS], F32, isOutput=True)
    r_out = nc.declare_dram_parameter("r_out", [H, 2 * NRESP], F32,
                                      isOutput=True)

    hb = nc.dram_tensor("hb", [NROWS, H], BF16)
    cb = nc.dram_tensor("cb", [NROWS, H], BF16)
    gfb = nc.dram_tensor("gfb", [NROWS, H], BF16)
    wxfb = nc.dram_tensor("wxfb", [NROWS, H], BF16)
    agi = nc.dram_tensor("agi", [MAXROOTS, 3 * H], BF16)
    ago = nc.dram_tensor("ago", [NCORES * MAXROOTS, 3 * H], BF16,
                         addr_space="Shared")

    def fm_cols(t, a, b2):
        """feature-major cols [a:b2) of t as a [128, 8, b2-a] AP."""
        return t[:, a:b2].rearrange("(of p) w -> p of w", p=128)

    with tile.TileContext(nc) as tc:
        with (
            tc.tile_pool(name="const", bufs=1) as cpool,
            tc.tile_pool(name="wxp", bufs=1) as wxp,
            tc.tile_pool(name="sp", bufs=1) as sp,
            tc.tile_pool(name="ps", bufs=1, space="PSUM") as ps,
        ):
            ident = cpool.tile([128, 128], F32, name="ident")
            make_identity(nc, ident[:])
            idx_sb = cpool.tile([128, s["icols"]], I16, name="idx_sb")
            nc.sync.dma_start(idx_sb[:], IDX[:])
            ss_sb = cpool.tile([128, s["scols"]], F32, name="ss_sb")
            nc.sync.dma_start(ss_sb[:], SS[:])
            s16_sb = cpool.tile([128, s["scols"]], BF16, name="s16_sb")
            nc.scalar.dma_start(s16_sb[:], S16[:])
            ut_sb = cpool.tile([128, KCH, 4096], BF16, name="ut_sb")
            nc.sync.dma_start(
                ut_sb[:, :4, :],
                UT[:512, :].rearrange("(k p) j -> p k j", p=128))
            nc.scalar.dma_start(
                ut_sb[:, 4:, :],
                UT[512:, :].rearrange("(k p) j -> p k j", p=128))
            zrow = cpool.tile([1, H], BF16, name="zrow")
            nc.vector.memset(zrow[:], 0.0)
            for t in (hb, cb, gfb, wxfb):
                nc.sync.dma_start(t[NROWS - 1:NROWS, :], zrow[:])

            def umm(pst, g, of, rhs_fn):
                blk = (g * 8 + of) * 128
                for k in range(KCH):
                    nc.tensor.matmul(pst, ut_sb[:, k, blk:blk + 128],
                                     rhs_fn(k), start=(k == 0),
                                     stop=(k == KCH - 1))
                return pst

            # persistent wx tiles (sweep B results) + resid tiles
            wx_sb = [wxp.tile([128, 8, BCOLS], F32, name=f"wx{g}")
                     for g in range(3)]
            wxres = [wxp.tile([128, 8, NRESP], F32, name=f"wxres{g}")
                     for g in range(3)]
            for g in range(3):
                nc.vector.memset(wxres[g][:], 0.0)
            st16 = []
            for nm3 in ("h", "c", "g"):
                t3 = wxp.tile([128, 8, 128], BF16, name=f"{nm3}16p")
                nc.gpsimd.memset(t3[:], 0.0)
                st16.append(t3)
            hsr = wxp.tile([128, 8, NRESP], F32, name="hsr")
            fcr = wxp.tile([128, 8, NRESP], F32, name="fcr")
            wxfres = wxp.tile([128, 8, NRESP], F32, name="wxfres")

            def gates(of, ps_i, ps_o, ps_u, wx_ap, fcs_of, hall, call,
                      h16_dst, c16_dst=None, out_rng=None):
                """Gate tail for one feature block into hall/call slices."""
                W = call.shape[2]
                i_t = sp.tile([128, W], F32, tag="i_t", name="i_t")
                o_t = sp.tile([128, W], F32, tag="o_t", name="o_t")
                u_t = sp.tile([128, W], F32, tag="u_t", name="u_t")
                if wx_ap is not None:
                    for dst, src, wx, fn in ((i_t, ps_i, wx_ap[0], SIG),
                                             (o_t, ps_o, wx_ap[1], SIG),
                                             (u_t, ps_u, wx_ap[2], TANH)):
                        pre = sp.tile([128, W], F32, tag="pre", name="pre", bufs=2)
                        nc.vector.tensor_add(pre[:], src, wx)
                        nc.scalar.activation(dst[:], pre[:], fn)
                else:
                    nc.scalar.activation(i_t[:], ps_i, SIG)
                    nc.scalar.activation(o_t[:], ps_o, SIG)
                    nc.scalar.activation(u_t[:], ps_u, TANH)
                nc.vector.tensor_mul(call[:, of, :], i_t[:], u_t[:])
                if fcs_of is not None:
                    nc.vector.tensor_add(call[:, of, :], call[:, of, :],
                                         fcs_of)
                th = sp.tile([128, W], F32, tag="pre", name="th", bufs=2)
                nc.scalar.activation(th[:], call[:, of, :], TANH)
                nc.vector.tensor_mul(hall[:, of, :], o_t[:], th[:])
                nc.vector.tensor_copy(h16_dst, hall[:, of, :])
                if c16_dst is not None:
                    nc.vector.tensor_copy(c16_dst, call[:, of, :])
                if out_rng is not None:
                    of2, a2, b2 = out_rng
                    nc.sync.dma_start(
                        h_out[of2 * 128:(of2 + 1) * 128, a2:b2],
                        hall[:, of, :])
                    nc.scalar.dma_start(
                        c_out[of2 * 128:(of2 + 1) * 128, a2:b2],
                        call[:, of, :])

            ecnt = [0]

            def cpcopy(out, in_):
                ecnt[0] += 1
                if ecnt[0] % 2:
                    nc.vector.tensor_copy(out, in_)
                else:
                    nc.scalar.copy(out, in_)

            def nm_store(src16, dram, s0, W):
                """src16: [128, 8, cgs*128] bf16 -> node-major bf16 rows."""
                for cg in range((W + 127) // 128):
                    cw = min(128, W - cg * 128)
                    asm = sp.tile([128, H], BF16, tag="asm", name="asm",
                                  bufs=2)
                    for of in range(8):
                        ecnt[0] += 1
                        eng = nc.sync if ecnt[0] % 2 else nc.scalar
                        eng.dma_start_transpose(
                            asm[:, of * 128:(of + 1) * 128],
                            src16[:, of, cg * 128:(cg + 1) * 128])
                    r0 = s0 + cg * 128
                    ecnt[0] += 1
                    eng = nc.sync if ecnt[0] % 2 else nc.scalar
                    eng.dma_start(dram[r0:r0 + cw, :], asm[:cw, :])

            # ------- Wx sweeps: A (L0 cols + gates) then gf, then B ----
            W0P = -(-W0 // 128) * 128
            BCP = -(-BCOLS // 128) * 128
            wxph_cm = tc.tile_pool(name="wxph", bufs=1)
            wxph = wxph_cm.__enter__()

            h16L = wxph.tile([128, 8, W0P], BF16, tag="h16L", name="h16L")
            c16L = wxph.tile([128, 8, W0P], BF16, tag="c16L", name="c16L")
            g16L = wxph.tile([128, 8, W0P], BF16, tag="g16L", name="g16L")
            if W0 < W0P:
                nc.gpsimd.memset(h16L[:, :, W0:], 0.0)
                nc.gpsimd.memset(c16L[:, :, W0:], 0.0)
                nc.gpsimd.memset(g16L[:, :, W0:], 0.0)
            xt_sb = wxph.tile([128, KCHX, NCOLS], F32R, name="xt_sb")
            nc.sync.dma_start(
                xt_sb[:, :5, :],
                xT[:5 * 128, :].rearrange("(k p) j -> p k j", p=128))
            nc.scalar.dma_start(
                xt_sb[:, 5:, :],
                xT[5 * 128:, :].rearrange("(k p) j -> p k j", p=128))


            # ---- fused sweeps per of: A-iou + L0 gates, B f/iou ----
            for of in range(8):
                wofA = wxph.tile([128, 5, 512], F32R, tag="wof", name="wofA",
                                 bufs=3)
                nc.sync.dma_start(
                    wofA[:], WT[:5 * 128, of * 512:(of + 1) * 512].rearrange(
                        "(k p) j -> p k j", p=128))
                wofB = wxph.tile([128, 5, 512], F32R, tag="wof", name="wofB",
                                 bufs=3)
                nc.scalar.dma_start(
                    wofB[:, :4, :],
                    WT[5 * 128:, of * 512:(of + 1) * 512].rearrange(
                        "(k p) j -> p k j", p=128))

                def wsl(k, g):
                    if k < 5:
                        return wofA[:, k, g * 128:g * 128 + 128]
                    return wofB[:, k - 5, g * 128:g * 128 + 128]

                pst = [ps.tile([128, W0], F32, tag=f"G{g}", name=f"psA{g}")
                       for g in range(3)]
                for k in range(KCHX):
                    for g in range(3):
                        nc.tensor.matmul(pst[g][:], wsl(k, g),
                                         xt_sb[:, k, 0:W0],
                                         start=(k == 0), stop=(k == KCHX - 1))
                hall0 = sp.tile([128, 1, W0], F32, tag="fcs",
                                name="hall0")
                call0 = sp.tile([128, 1, W0], F32, tag="hs16",
                                name="call0")
                gates(0, pst[0][:], pst[1][:], pst[2][:],
                      None, None, hall0, call0, h16L[:, of, :W0],
                      c16_dst=c16L[:, of, :W0], out_rng=(of, 0, W0))
                # B: f gate then iou
                pBf = ps.tile([128, BCOLS], F32, tag="TR", name="psBf",
                              bufs=2)
                for k in range(KCHX):
                    nc.tensor.matmul(pBf[:], wsl(k, 3), xt_sb[:, k, W0:NCOLS],
                                     start=(k == 0), stop=(k == KCHX - 1))
                wf16 = wxph.tile([128, BCOLS], BF16, tag="wf16",
                                 name="wf16", bufs=2)
                nc.vector.tensor_copy(wf16[:], pBf[:])
                nc.scalar.dma_start(
                    wxfb[W0:NCOLS, of * 128:(of + 1) * 128].rearrange(
                        "w p -> p w"), wf16[:])
                for g in range(3):
                    pBi = ps.tile([128, BCOLS], F32, tag=f"G{g}", name="psBi")
                    for k in range(KCHX):
                        nc.tensor.matmul(pBi[:], wsl(k, g),
                                         xt_sb[:, k, W0:NCOLS],
                                         start=(k == 0), stop=(k == KCHX - 1))
                    nc.vector.tensor_copy(wx_sb[g][:, of, :], pBi[:])
                    nc.scalar.copy(wxres[g][:, of, 0:NRES],
                                   pBi[:, ROFF:ROFF + NRES])
            nm_store(h16L, hb, 0, W0)
            nm_store(c16L, cb, 0, W0)
            # ---- L0 gf ----
            for of in range(8):
                p = ps.tile([128, W0], F32, tag="G0", name="psgf0")
                umm(p[:], 3, of, lambda k: h16L[:, k, :W0])
                nc.vector.tensor_copy(g16L[:, of, :W0], p[:])
            nm_store(g16L, gfb, 0, W0)
            wxph_cm.__exit__(None, None, None)
            gp_cm = tc.tile_pool(name="gp", bufs=1)
            gp = gp_cm.__enter__()

            # ---------------- forest levels ----------------
            for lv in s["levels"]:
                L, W, s0, nch = lv["L"], lv["W"], lv["s0"], lv["nch"]
                iA, iB, sc = lv["iA"], lv["iB"], lv["sc"]

                GH = gp.tile([128, nch, H], BF16, tag="GH", name="GH", bufs=2)
                nc.gpsimd.dma_gather(
                    out_ap=GH[:], in_ap=hb[:, :],
                    idxs_ap=idx_sb[:, iA:iA + nch * 8],
                    num_idxs=nch * 128, num_idxs_reg=nch * 128,
                    elem_size=H)
                GC = gp.tile([128, nch, H], BF16, tag="GC", name="GC", bufs=2)
                nc.gpsimd.dma_gather(
                    out_ap=GC[:], in_ap=cb[:, :],
                    idxs_ap=idx_sb[:, iA:iA + nch * 8],
                    num_idxs=nch * 128, num_idxs_reg=nch * 128, elem_size=H)
                GW = gp.tile([128, nch, H], BF16, tag="GW", name="GW", bufs=2)
                nc.gpsimd.dma_gather(
                    out_ap=GW[:], in_ap=wxfb[:, :],
                    idxs_ap=idx_sb[:, iB:iB + nch * 8],
                    num_idxs=nch * 128, num_idxs_reg=nch * 128, elem_size=H)
                GG = gp.tile([128, nch, H], BF16, tag="GG", name="GG", bufs=2)
                nc.gpsimd.dma_gather(
                    out_ap=GG[:], in_ap=gfb[:, :],
                    idxs_ap=idx_sb[:, iA:iA + nch * 8],
                    num_idxs=nch * 128, num_idxs_reg=nch * 128, elem_size=H)
                php = ps.tile([128, 8, 128], F32, tag="G2", name="php")
                for fc in range(8):
                    for c2 in range(nch):
                        nc.tensor.matmul(
                            php[:, fc, :W], GH[:, c2, fc * 128:(fc + 1) * 128],
                            s16_sb[:, sc + c2 * W:sc + (c2 + 1) * W],
                            start=(c2 == 0), stop=(c2 == nch - 1))
                hs16 = sp.tile([128, 8, W], BF16, tag="hs16", name="hs16")
                nc.vector.tensor_copy(hs16[:], php[:, :, :W])
                T16 = gp.tile([128, nch, H], BF16, tag="T16", name="T16",
                              bufs=2)
                nc.vector.tensor_add(T16[:], GG[:], GW[:])
                nc.scalar.activation(T16[:], T16[:], SIG)
                T1 = gp.tile([128, nch, H], F32, tag="T1", name="T1")
                nc.vector.tensor_mul(T1[:], T16[:], GC[:])
                fcp = ps.tile([128, 8, 128], F32, tag="G1", name="fcp")
                for fc in range(8):
                    for c2 in range(nch):
                        nc.tensor.matmul(
                            fcp[:, fc, :W], T1[:, c2, fc * 128:(fc + 1) * 128],
                            ss_sb[:, sc + c2 * W:sc + (c2 + 1) * W],
                            start=(c2 == 0), stop=(c2 == nch - 1))
                fcs = sp.tile([128, 8, W], F32, tag="fcs", name="fcs")
                nc.vector.tensor_copy(fcs[:], fcp[:, :, :W])
                hall = sp.tile([128, 8, W], F32, tag="hall", name="hall")
                call = sp.tile([128, 8, W], F32, tag="call", name="call")
                pg0 = ps.tile([128, 8, 128], F32, tag="G0", name="pg0")
                for of in range(8):
                    umm(pg0[:, of, :W], 0, of, lambda k: hs16[:, k, :])
                pg1 = ps.tile([128, 8, 128], F32, tag="G1", name="pg1")
                for of in range(8):
                    umm(pg1[:, of, :W], 1, of, lambda k: hs16[:, k, :])
                pg2 = ps.tile([128, 8, 128], F32, tag="G2", name="pg2")
                for of in range(8):
                    umm(pg2[:, of, :W], 2, of, lambda k: hs16[:, k, :])
                woff = s0 - W0
                i_t = sp.tile([128, 8, W], F32, tag="i_t", name="i_t")
                nc.vector.tensor_add(i_t[:], pg0[:, :, :W],
                                     wx_sb[0][:, :, woff:woff + W])
                nc.scalar.activation(i_t[:], i_t[:], SIG)
                o_t = sp.tile([128, 8, W], F32, tag="o_t", name="o_t")
                nc.vector.tensor_add(o_t[:], pg1[:, :, :W],
                                     wx_sb[1][:, :, woff:woff + W])
                nc.scalar.activation(o_t[:], o_t[:], SIG)
                u_t = sp.tile([128, 8, W], F32, tag="u_t", name="u_t")
                nc.vector.tensor_add(u_t[:], pg2[:, :, :W],
                                     wx_sb[2][:, :, woff:woff + W])
                nc.scalar.activation(u_t[:], u_t[:], TANH)
                nc.vector.tensor_mul(call[:], i_t[:], u_t[:])
                nc.vector.tensor_add(call[:], call[:], fcs[:])
                th = sp.tile([128, 8, W], F32, tag="i_t", name="th")
                nc.scalar.activation(th[:], call[:], TANH)
                nc.vector.tensor_mul(hall[:], o_t[:], th[:])
                h16, c16, g16 = st16
                nc.vector.tensor_copy(h16[:, :, :W], hall[:])
                nc.vector.tensor_copy(c16[:, :, :W], call[:])
                nm_store(h16, hb, s0, W)
                nm_store(c16, cb, s0, W)
                nc.sync.dma_start(fm_cols(h_out, s0, s0 + W), hall[:])
                nc.sync.dma_start(fm_cols(c_out, s0, s0 + W), call[:])
                pgf = ps.tile([128, 8, 128], F32, tag="G0", name="pgf")
                for of in range(8):
                    umm(pgf[:, of, :W], 3, of, lambda k: h16[:, k, :W])
                nc.vector.tensor_copy(g16[:, :, :W], pgf[:, :, :W])
                nm_store(g16, gfb, s0, W)

            # ---------------- roots -> AG ----------------
            ir = s["iroots"]
            for nm2, src2, off in (("grh", hb, 0), ("grc", cb, H),
                                   ("grg", gfb, 2 * H)):
                GR = gp.tile([128, 1, H], BF16, tag="GR", name=nm2)
                nc.gpsimd.dma_gather(
                    out_ap=GR[:], in_ap=src2[:, :],
                    idxs_ap=idx_sb[:, ir:ir + 8],
                    num_idxs=128, num_idxs_reg=128, elem_size=H)
                nc.sync.dma_start(agi[:, off:off + H], GR[:MAXROOTS, 0, :])
            nc.gpsimd.collective_compute(
                "AllGather", mybir.AluOpType.bypass,
                replica_groups=[list(range(NCORES))],
                ins=[agi[:]], outs=[ago[:]])

            # ---------------- residual ----------------
            ip = s["ipair"]

            def pair_gather(nm, tag, off):
                G16 = gp.tile([128, 1, H], BF16, tag="GW16",
                              name=nm + "16", bufs=2)
                nc.gpsimd.dma_gather(
                    out_ap=G16[:], in_ap=ago[:, off:off + H],
                    idxs_ap=idx_sb[:, ip:ip + 8],
                    num_idxs=128, num_idxs_reg=128, elem_size=H,
                    elem_step=3 * H)
                G = gp.tile([128, 1, H], F32, tag=tag, name=nm, bufs=2)
                nc.vector.tensor_copy(G[:], G16[:])
                return G

            PRH = pair_gather("PRH", "GH", 0)
            PRC = pair_gather("PRC", "GC", H)
            PRG = pair_gather("PRG", "GG", 2 * H)
            PRW16 = gp.tile([128, 1, H], BF16, tag="GW16", name="PRW16",
                            bufs=2)
            nc.gpsimd.dma_gather(
                out_ap=PRW16[:], in_ap=wxfb[:, :],
                idxs_ap=idx_sb[:, ip + 8:ip + 16],
                num_idxs=128, num_idxs_reg=128, elem_size=H)
            PRW = gp.tile([128, 1, H], F32, tag="GW", name="PRW", bufs=2)
            nc.vector.tensor_copy(PRW[:], PRW16[:])
            nc.vector.tensor_add(PRG[:], PRG[:], PRW[:])
            nc.scalar.activation(PRG[:], PRG[:], SIG)
            nc.vector.tensor_mul(PRG[:], PRG[:], PRC[:])
            spc = s["spcol"]
            for fc in range(8):
                p = ps.tile([128, NRESP], F32, tag="TR", name="pshr", bufs=2)
                nc.tensor.matmul(p[:], PRH[:, 0, fc * 128:(fc + 1) * 128],
                                 ss_sb[:, spc:spc + NRESP],
                                 start=True, stop=True)
                nc.vector.tensor_copy(hsr[:, fc, :], p[:])
                p2 = ps.tile([128, NRESP], F32, tag="TR", name="psfr", bufs=2)
                nc.tensor.matmul(p2[:], PRG[:, 0, fc * 128:(fc + 1) * 128],
                                 ss_sb[:, spc:spc + NRESP],
                                 start=True, stop=True)
                nc.vector.tensor_copy(fcr[:, fc, :], p2[:])
            # wxf of resid nodes, feature-major blocks
            RW16 = gp.tile([128, 1, H], BF16, tag="GW16", name="RW16",
                           bufs=2)
            nc.gpsimd.dma_gather(
                out_ap=RW16[:], in_ap=wxfb[:, :],
                idxs_ap=idx_sb[:, s["irs"]:s["irs"] + 8],
                num_idxs=128, num_idxs_reg=128, elem_size=H)
            RW = gp.tile([128, 1, H], F32, tag="GW", name="RW", bufs=2)
            nc.vector.tensor_copy(RW[:], RW16[:])
            for fc in range(8):
                pt = ps.tile([128, 128], F32, tag="TR", name="ptw",
                             bufs=2)
                nc.tensor.transpose(pt[:], RW[:, 0, fc * 128:(fc + 1) * 128],
                                    ident[:])
                nc.scalar.copy(wxfres[:, fc, :], pt[:, :NRESP])

            for sub in s["subs"]:
                r0, w = sub["r0"], sub["w"]
                hs16s = sp.tile([128, 8, w], BF16, tag="hs16s", name="hs16s")
                nc.vector.tensor_copy(hs16s[:], hsr[:, :, r0:r0 + w])
                psg = []
                for g in range(3):
                    p = ps.tile([128, 8, w], F32, tag=f"G{g}", name="psR")
                    for of in range(8):
                        umm(p[:, of, :], g, of, lambda k: hs16s[:, k, :])
                    psg.append(p)
                i_t = sp.tile([128, 8, w], F32, tag="ri", name="ri")
                nc.vector.tensor_add(i_t[:], psg[0][:],
                                     wxres[0][:, :, r0:r0 + w])
                nc.scalar.activation(i_t[:], i_t[:], SIG)
                o_t = sp.tile([128, 8, w], F32, tag="ro", name="ro")
                nc.vector.tensor_add(o_t[:], psg[1][:],
                                     wxres[1][:, :, r0:r0 + w])
                nc.scalar.activation(o_t[:], o_t[:], SIG)
                u_t = sp.tile([128, 8, w], F32, tag="ru", name="ru")
                nc.vector.tensor_add(u_t[:], psg[2][:],
                                     wxres[2][:, :, r0:r0 + w])
                nc.scalar.activation(u_t[:], u_t[:], TANH)
                c_t = sp.tile([128, 8, w], F32, tag="rc", name="rc")
                nc.vector.tensor_mul(c_t[:], i_t[:], u_t[:])
                nc.vector.tensor_add(c_t[:], c_t[:], fcr[:, :, r0:r0 + w])
                th = sp.tile([128, 8, w], F32, tag="rth", name="rth")
                nc.scalar.activation(th[:], c_t[:], TANH)
                h_t = sp.tile([128, 8, w], F32, tag="rh", name="rh")
                nc.vector.tensor_mul(h_t[:], o_t[:], th[:])
                nc.scalar.dma_start(fm_cols(r_out, r0, r0 + w), h_t[:])
                nc.scalar.dma_start(
                    fm_cols(r_out, NRESP + r0, NRESP + r0 + w), c_t[:])
                if sub["edges"]:
                    h16s = sp.tile([128, 8, w], BF16, tag="h16s", name="h16s")
                    nc.vector.tensor_copy(h16s[:], h_t[:])
                    pf = ps.tile([128, 8, w], F32, tag="G0", name="psRf")
                    for of in range(8):
                        umm(pf[:, of, :], 3, of, lambda k: h16s[:, k, :])
                    for (jp, jk) in sub["edges"]:
                        j = jk - r0
                        nc.vector.tensor_add(hsr[:, :, jp:jp + 1],
                                             hsr[:, :, jp:jp + 1],
                                             h_t[:, :, j:j + 1])
                        e1 = sp.tile([128, 8, 1], F32, tag="e1", name="e1")
                        nc.vector.tensor_add(e1[:], pf[:, :, j:j + 1],
                                             wxfres[:, :, jp:jp + 1])
                        nc.scalar.activation(e1[:], e1[:], SIG)
                        nc.vector.tensor_mul(e1[:], e1[:], c_t[:, :, j:j + 1])
                        nc.vector.tensor_add(fcr[:, :, jp:jp + 1],
                                             fcr[:, :, jp:jp + 1], e1[:])
            gp_cm.__exit__(None, None, None)

    nc.finalize()
    return nc


def prepare(x=None, head=None, **kw):
    x = np.asarray(x, np.float32)
    s = _schedule(np.asarray(head))

    Ws = {g: np.asarray(kw[f"W_{g}"], np.float32) for g in "iouf"}
    Us = {g: np.asarray(kw[f"U_{g}"], np.float32) for g in "iouf"}
    bs = {g: np.asarray(kw[f"b_{g}"], np.float32) for g in "iouf"}

    WT = np.zeros((KCHX * 128, 4096), np.float32)
    UTf = np.zeros((KCH * 128, 4096), np.float32)
    for gi, g in enumerate("iouf"):
        for of in range(8):
            wblk = of * 512 + gi * 128                      # of-major
            ublk = (gi * 8 + of) * 128                      # g-major
            WT[:H, wblk:wblk + 128] = Ws[g][of * 128:(of + 1) * 128, :].T
            WT[H, wblk:wblk + 128] = bs[g][of * 128:(of + 1) * 128]
            UTf[:, ublk:ublk + 128] = Us[g][of * 128:(of + 1) * 128, :].T
    from ml_dtypes import bfloat16
    UT = UTf.astype(bfloat16)

    NCOLS = s["NCOLS"]
    in_maps = []
    for b in range(NCORES):
        xTb = np.zeros((KCHX * 128, NCOLS), np.float32)
        ids = s["core_nodes"][b]
        valid = ids >= 0
        xTb[:H, :s["NSLOT"]][:, valid] = x[ids[valid]].T
        xTb[:H, s["NSLOT"]:NCOLS] = x[s["resid_order"]].T
        xTb[H, :] = 1.0
        in_maps.append({
            "xT": xTb, "WT": WT, "UT": UT,
            "IDX": np.ascontiguousarray(s["idxt"][b]),
            "SS": np.ascontiguousarray(s["sst"][b]),
            "S16": np.ascontiguousarray(s["sst"][b]).astype(bfloat16),
        })

    nc = _build_nc(s)

    def post(results):
        h = np.zeros((N, H), np.float32)
        c = np.zeros((N, H), np.float32)
        for b in range(NCORES):
            ids = s["core_nodes"][b]
            valid = np.where(ids >= 0)[0]
            h[ids[valid]] = results[b]["h_out"][:, valid].T
            c[ids[valid]] = results[b]["c_out"][:, valid].T
        r = results[0]["r_out"]
        h[s["resid_order"]] = r[:, :s["NRES"]].T
        c[s["resid_order"]] = r[:, NRESP:NRESP + s["NRES"]].T
        return h, c

    return nc, in_maps, post


def kernel(x=None, head=None, **kw):
    import concourse.mybir as mybir  # noqa: F401  (env check)
    from concourse.bass_utils import run_bass_kernel_spmd

    nc, in_maps, post = prepare(x=x, head=head, **kw)
    res = run_bass_kernel_spmd(nc, in_maps, list(range(NCORES)))
    return post(res.results)
